# revision 10
# baseline (speedup 1.0000x reference)
"""Trainium2 Bass kernel v2 for nn_ActorBatchNet (Set2Set + torsion MLP).

Data-parallel over graphs: 8 cores x 256 graphs. Key optimizations over v1:
- fp16 matmul operands everywhere (1 cy/row + single-pass LDWEIGHTS vs 4 cy/row
  double-pumped fp32)
- all torsion gathers issued up front (overlap with Set2Set on the PE)
- fp16 gather rows (256B) halve gather DMA traffic
- half-split Set2Set pipeline so DVE/Act softmax+LSTM work hides under PE
- phase C: 4-tile groups, batched W1 matmuls (512-col streams), ones-matrix
  matmul for partition-broadcast segment sums

The gather stream (256 x 128-row indirect SWDGE DMAs at ~1.4us each) is the
measured critical path: descriptor generation on the single Pool engine paces
at ~11ns/row regardless of batching mechanism (verified against the custom
InstDMAGatherAnt path, single/multi-packet modes, multi-queue, and staging+
re-gather architectures - all hit the same ~8-11ns/row Q7 floor).
"""

import sys

for _p in ("/opt/trn_rl_repo", "/root/.axon_site/_ro/trn_rl_repo"):
    if _p not in sys.path:
        sys.path.insert(0, _p)

import numpy as np

import concourse.bass as bass
import concourse.bacc as bacc
import concourse.mybir as mybir
from concourse.tile import TileContext
from concourse.bass_utils import run_bass_kernel_spmd


def _ensure_ntff_hook():
    """This image's `antenv` lacks `axon_hooks`; if the caller sets BASS_TRACE,
    run_bass_kernel_spmd would crash on the import. Shim it (no-op where the
    real module exists)."""
    try:
        import antenv.axon_hooks  # noqa: F401
        return
    except ImportError:
        pass
    try:
        import os
        import types

        import antenv

        _h = [None]
        m = types.ModuleType("antenv.axon_hooks")
        m.set_axon_ntff_profile_hook = lambda hook: _h.__setitem__(0, hook)
        m.get_axon_ntff_profile_hook = lambda: _h[0]
        sys.modules["antenv.axon_hooks"] = m
        antenv.axon_hooks = m
        boot = "/root/.axon_site/trn_agent_boot/trn_boot.py"
        so = "/opt/axon/libaxon_pjrt.so"
        if os.path.exists(boot) and os.path.exists(so):
            if "/root/.axon_site" not in sys.path:
                sys.path.insert(0, "/root/.axon_site")
            from trn_agent_boot.trn_boot import _ntff_profile_via_ctypes

            _h[0] = _ntff_profile_via_ctypes(so)
    except Exception:
        pass


_ensure_ntff_hook()

F32 = mybir.dt.float32
F16 = mybir.dt.float16
I32 = mybir.dt.int32
AF = mybir.ActivationFunctionType

G = 2048
NODES_PER_G = 64
DIM = 128
TORS_PER_G = 32
ACTD = 36
STEPS = 6
MAX_T = 32
NC = 8
NG = G // NC                # 256 graphs/core
NN = NG * NODES_PER_G       # 16384 nodes/core
NT = NG * TORS_PER_G        # 8192 torsions/core
NTILE = NN // 128           # 128 node tiles
TTILE = NT // 128           # 64 torsion tiles
P = 128
HALF = NTILE // 2           # 64 node tiles per half
GRP = 16                    # phase C groups of 4 torsion tiles

LAST = None
_CACHED = None


def build_bass():
    nc = bacc.Bacc("TRN2", target_bir_lowering=False, debug=False)

    # ---- DRAM parameters (per core) ----
    xT16 = nc.declare_dram_parameter("xT16", [P, NN], F16, isOutput=False)
    xrm16 = nc.declare_dram_parameter("xrm16", [P, NN], F16, isOutput=False)
    xfull16 = nc.declare_dram_parameter("xfull16", [G * NODES_PER_G, DIM], F16, isOutput=False)
    idx4 = nc.declare_dram_parameter("idx4", [P, 4 * TTILE], I32, isOutput=False)
    wA16 = nc.declare_dram_parameter("wA16", [DIM, 4 * DIM], F16, isOutput=False)
    wB16 = nc.declare_dram_parameter("wB16", [DIM, 4 * DIM], F16, isOutput=False)
    bs4 = nc.declare_dram_parameter("bs4", [P, 4], F32, isOutput=False)
    expb = nc.declare_dram_parameter("expb", [P, 2], F32, isOutput=False)
    fcq16 = nc.declare_dram_parameter("fcq16", [DIM, DIM], F16, isOutput=False)
    fcr16 = nc.declare_dram_parameter("fcr16", [DIM, DIM], F16, isOutput=False)
    fcb = nc.declare_dram_parameter("fcb", [P, 1], F32, isOutput=False)
    w1g16 = nc.declare_dram_parameter("w1g16", [DIM, DIM], F16, isOutput=False)
    w1x16 = nc.declare_dram_parameter("w1x16", [P, 4 * DIM], F16, isOutput=False)
    b1 = nc.declare_dram_parameter("b1", [P, 1], F32, isOutput=False)
    w2T16 = nc.declare_dram_parameter("w2T16", [DIM, ACTD], F16, isOutput=False)
    b2t = nc.declare_dram_parameter("b2t", [P, ACTD], F32, isOutput=False)
    ones16d = nc.declare_dram_parameter("ones16d", [P, P], F16, isOutput=False)
    ident16 = nc.declare_dram_parameter("ident16", [P, P], F16, isOutput=False)
    out = nc.declare_dram_parameter("out", [NT, ACTD], F32, isOutput=True)

    with TileContext(nc) as tc:
        with tc.tile_pool(name="pc", bufs=1) as pc:
            # ---- persistent SBUF ----
            xT_sb = pc.tile([P, NN], F16, tag="xT")
            xrm_sb = pc.tile([P, NN], F16, tag="xrm")
            gat_all = pc.tile([P, TTILE * 512], F16, tag="gat")
            idx_sb = pc.tile([P, 4 * TTILE], I32, tag="idx")
            wA_sb = pc.tile([P, 4 * DIM], F16, tag="wA")
            wB_sb = pc.tile([P, 4 * DIM], F16, tag="wB")
            bs_sb = pc.tile([P, 4], F32, tag="bs")
            expb_sb = pc.tile([P, 2], F32, tag="expb")
            fcq_sb = pc.tile([P, DIM], F16, tag="fcq")
            fcr_sb = pc.tile([P, DIM], F16, tag="fcr")
            fcb_sb = pc.tile([P, 1], F32, tag="fcb")
            w1g_sb = pc.tile([P, DIM], F16, tag="w1g")
            w1x_sb = pc.tile([P, 4 * DIM], F16, tag="w1x")
            b1_sb = pc.tile([P, 1], F32, tag="b1")
            w2_sb = pc.tile([P, ACTD], F16, tag="w2")
            b2_sb = pc.tile([P, ACTD], F32, tag="b2")
            ones16_sb = pc.tile([P, P], F16, tag="ones16")
            id_sb = pc.tile([P, P], F16, tag="id")
            # Set2Set state (hT16 holds 2h, cS holds 2c)
            hT16 = pc.tile([P, 2 * NTILE], F16, tag="hT16")
            rT16 = pc.tile([P, 2 * NTILE], F16, tag="rT16")
            cS = pc.tile([P, 2 * NTILE], F32, tag="cS")
            ti = pc.tile([P, 2 * NTILE], F32, tag="ti")
            tf = pc.tile([P, 2 * NTILE], F32, tag="tf")
            tg = pc.tile([P, 2 * NTILE], F32, tag="tg")
            to = pc.tile([P, 2 * NTILE], F32, tag="to")
            tnc = pc.tile([P, 2 * NTILE], F32, tag="tnc")
            expe16 = pc.tile([P, 2 * NTILE], F16, tag="expe16")
            rb = pc.tile([P, 2 * NTILE], F32, tag="rb")
            geT16 = pc.tile([P, 2 * NTILE], F16, tag="geT16")
            hgT = pc.tile([P, 2 * NTILE], F32, tag="hgT")

            # ---- loads ----
            # the first idx chunk rides the Sync HWDGE queue, which starts
            # issuing ~6us before gpsimd clears its entry barrier + stage
            # drains — pulls the first gather (the critical-path stream head)
            # earlier; the rest loads on gpsimd and lands well before use
            # all idx chunks ride the Sync HWDGE queue: it starts ~6us before
            # gpsimd clears its entry barrier, and keeping the gpsimd queue
            # free of loads lets gather 0 issue the moment the barrier drops
            # (a gpsimd-queued load would delay the whole stream by its own
            # DMA completion)
            nc.sync.dma_start(out=idx_sb[:, 0:8], in_=idx4[:, 0:8])
            nc.sync.dma_start(out=idx_sb[:, 8:64], in_=idx4[:, 8:64])
            nc.sync.dma_start(out=idx_sb[:, 64:], in_=idx4[:, 64:])
            nc.sync.dma_start(out=wA_sb[:], in_=wA16[:, :])
            nc.sync.dma_start(out=wB_sb[:], in_=wB16[:, :])
            nc.sync.dma_start(out=bs_sb[:], in_=bs4[:, :])
            nc.sync.dma_start(out=expb_sb[:], in_=expb[:, :])
            nc.sync.dma_start(out=fcq_sb[:], in_=fcq16[:, :])
            nc.sync.dma_start(out=fcr_sb[:], in_=fcr16[:, :])
            nc.sync.dma_start(out=fcb_sb[:], in_=fcb[:, :])
            nc.sync.dma_start(out=w1g_sb[:], in_=w1g16[:, :])
            nc.sync.dma_start(out=w1x_sb[:], in_=w1x16[:, :])
            nc.sync.dma_start(out=b1_sb[:], in_=b1[:, :])
            nc.sync.dma_start(out=w2_sb[:], in_=w2T16[:, :])
            nc.sync.dma_start(out=b2_sb[:], in_=b2t[:, :])
            nc.sync.dma_start(out=ones16_sb[:], in_=ones16d[:, :])
            nc.sync.dma_start(out=id_sb[:], in_=ident16[:, :])
            # x loads split by half so the first e-pass can start earlier
            nc.sync.dma_start(out=xT_sb[:, :NN // 2], in_=xT16[:, :NN // 2])
            nc.sync.dma_start(out=xT_sb[:, NN // 2:], in_=xT16[:, NN // 2:])
            nc.sync.dma_start(out=xrm_sb[:, :NN // 2], in_=xrm16[:, :NN // 2])
            nc.sync.dma_start(out=xrm_sb[:, NN // 2:], in_=xrm16[:, NN // 2:])
            nc.vector.memset(hT16[:], 0.0)
            nc.vector.memset(rT16[:], 0.0)
            nc.vector.memset(cS[:], 0.0)

            # ---- all torsion gathers up front (overlap with Set2Set) ----
            for b in range(TTILE):
                for s in range(4):
                    nc.gpsimd.indirect_dma_start(
                        out=gat_all[:, b * 512 + s * P: b * 512 + (s + 1) * P],
                        out_offset=None,
                        in_=xfull16[:, :],
                        in_offset=bass.IndirectOffsetOnAxis(
                            ap=idx_sb[:, 4 * b + s:4 * b + s + 1], axis=0),
                    )

            # ---- Set2Set: 6 steps, half-split pipeline ----
            with tc.tile_pool(name="pg", bufs=2, space="PSUM") as pg, \
                 tc.tile_pool(name="pe_", bufs=2, space="PSUM") as ppe, \
                 tc.tile_pool(name="ps_", bufs=2, space="PSUM") as pps, \
                 tc.tile_pool(name="pr_", bufs=2, space="PSUM") as ppr:
                for step in range(STEPS):
                    for h in (0, 1):
                        ch = slice(h * P, (h + 1) * P)
                        # gates: psum [128, 512] = 4 gate blocks of 128 cols
                        gp = pg.tile([P, 4 * P], F32, tag="gp")
                        for k in range(4):
                            nc.tensor.matmul(out=gp[:, k * P:(k + 1) * P],
                                             lhsT=wA_sb[:, k * P:(k + 1) * P],
                                             rhs=hT16[:, ch], start=True, stop=False)
                            nc.tensor.matmul(out=gp[:, k * P:(k + 1) * P],
                                             lhsT=wB_sb[:, k * P:(k + 1) * P],
                                             rhs=rT16[:, ch], start=False, stop=True)
                        # LSTM pointwise, tanh-only (sigmoid(x) = (1+tanh(x/2))/2;
                        # hT16 holds 2h, cS holds 2c; wA/fcq pre-scaled by 0.5)
                        nc.scalar.activation(out=ti[:, ch], in_=gp[:, 0:P],
                                             func=AF.Tanh, scale=0.5, bias=bs_sb[:, 0:1])
                        nc.scalar.activation(out=tf[:, ch], in_=gp[:, P:2 * P],
                                             func=AF.Tanh, scale=0.5, bias=bs_sb[:, 1:2])
                        nc.scalar.activation(out=tg[:, ch], in_=gp[:, 2 * P:3 * P],
                                             func=AF.Tanh, bias=bs_sb[:, 2:3])
                        nc.scalar.activation(out=to[:, ch], in_=gp[:, 3 * P:4 * P],
                                             func=AF.Tanh, scale=0.5, bias=bs_sb[:, 3:4])
                        # S' = 0.5*(S + tf*S) + (G + ti*G)
                        nc.vector.tensor_mul(out=tf[:, ch], in0=tf[:, ch], in1=cS[:, ch])
                        nc.vector.tensor_add(out=tf[:, ch], in0=tf[:, ch], in1=cS[:, ch])
                        nc.vector.tensor_scalar_mul(out=tf[:, ch], in0=tf[:, ch],
                                                    scalar1=0.5)
                        nc.vector.tensor_mul(out=ti[:, ch], in0=ti[:, ch], in1=tg[:, ch])
                        nc.vector.tensor_add(out=ti[:, ch], in0=ti[:, ch], in1=tg[:, ch])
                        nc.vector.tensor_add(out=cS[:, ch], in0=tf[:, ch], in1=ti[:, ch])
                        nc.scalar.activation(out=tnc[:, ch], in_=cS[:, ch],
                                             func=AF.Tanh, scale=0.5)
                        # H' = tnc + to*tnc  (= 2h)
                        nc.vector.tensor_mul(out=to[:, ch], in0=to[:, ch], in1=tnc[:, ch])
                        nc.vector.tensor_add(out=hT16[:, ch], in0=tnc[:, ch], in1=to[:, ch])

                    pes = []
                    for h in (0, 1):
                        # e scores for this half's 64 node tiles (pe = 2e)
                        pe = ppe.tile([P, P], F32, tag="pe")
                        for t in range(HALF):
                            tt = h * HALF + t
                            nc.tensor.matmul(out=pe[:, 2 * t:2 * t + 2],
                                             lhsT=xT_sb[:, tt * P:(tt + 1) * P],
                                             rhs=hT16[:, 2 * tt:2 * tt + 2],
                                             start=True, stop=True)
                        pes.append(pe)
                        ch = slice(h * P, (h + 1) * P)
                        # fused mask+exp: expe16 = exp(e - 11), parity-split bias
                        for par in (0, 1):
                            nc.scalar.activation(
                                out=expe16[:, ch].rearrange(
                                    "p (t two) -> p t two", two=2)[:, :, par:par + 1],
                                in_=pe[:].rearrange(
                                    "p (t two) -> p t two", two=2)[:, :, par:par + 1],
                                func=AF.Exp, scale=0.5,
                                bias=expb_sb[:, par:par + 1])
                    for h in (0, 1):
                        ch = slice(h * P, (h + 1) * P)
                        # segment sums broadcast to all partitions via ones matmul
                        sm = pps.tile([P, P], F32, tag="sm")
                        nc.tensor.matmul(out=sm[:], lhsT=ones16_sb[:],
                                         rhs=expe16[:, ch], start=True, stop=True)
                        nc.vector.reciprocal(out=rb[:, ch], in_=sm[:])
                        # unnormalized r, then normalize into rT16
                        pr = ppr.tile([P, P], F32, tag="pr")
                        for t in range(HALF):
                            tt = h * HALF + t
                            nc.tensor.matmul(out=pr[:, 2 * t:2 * t + 2],
                                             lhsT=xrm_sb[:, tt * P:(tt + 1) * P],
                                             rhs=expe16[:, 2 * tt:2 * tt + 2],
                                             start=True, stop=True)
                        nc.vector.tensor_mul(out=rT16[:, ch], in0=pr[:], in1=rb[:, ch])

                # graph embed + per-graph hidden contribution
                ge_ps = ppe.tile([P, P], F32, tag="pe")
                ge_ps2 = ppe.tile([P, P], F32, tag="pe")
                for h, gep in ((0, ge_ps), (1, ge_ps2)):
                    ch = slice(h * P, (h + 1) * P)
                    nc.tensor.matmul(out=gep[:], lhsT=fcq_sb[:], rhs=hT16[:, ch],
                                     start=True, stop=False)
                    nc.tensor.matmul(out=gep[:], lhsT=fcr_sb[:], rhs=rT16[:, ch],
                                     start=False, stop=True)
                    nc.vector.tensor_scalar_add(out=geT16[:, ch], in0=gep[:],
                                                scalar1=fcb_sb[:, 0:1])
                hg_ps = ppr.tile([P, P], F32, tag="pr")
                hg_ps2 = ppr.tile([P, P], F32, tag="pr")
                for h, hgp in ((0, hg_ps), (1, hg_ps2)):
                    ch = slice(h * P, (h + 1) * P)
                    nc.tensor.matmul(out=hgp[:], lhsT=w1g_sb[:], rhs=geT16[:, ch],
                                     start=True, stop=True)
                    nc.vector.tensor_copy(out=hgT[:, ch], in_=hgp[:])

            # ---- MLP over torsions: groups of 4 tiles ----
            with tc.tile_pool(name="pzt", bufs=2) as pzt, \
                 tc.tile_pool(name="pzs", bufs=8) as pzs, \
                 tc.tile_pool(name="pzp", bufs=4, space="PSUM") as pzp, \
                 tc.tile_pool(name="phd", bufs=2, space="PSUM") as phd, \
                 tc.tile_pool(name="plg", bufs=2, space="PSUM") as plg:
                # 4-tile groups for W1-stream batching; the last 8 tiles run
                # singly so the post-last-gather tail chain is short
                groups = [(4 * g, 4) for g in range(14)] + \
                         [(56 + i, 1) for i in range(8)]
                for b0, nb in groups:
                    cw = nb * P
                    ztg = pzt.tile([P, 4 * 512], F16, tag="ztg")
                    for bb in range(nb):
                        b = b0 + bb
                        for s in range(4):
                            ztp = pzp.tile([P, P], F16, tag="ztp")
                            nc.tensor.transpose(
                                out=ztp[:],
                                in_=gat_all[:, b * 512 + s * P: b * 512 + (s + 1) * P],
                                identity=id_sb[:])
                            dst = ztg[:, s * cw + bb * P: s * cw + (bb + 1) * P]
                            if (bb * 4 + s) % 2 == 0:
                                nc.vector.tensor_copy(out=dst, in_=ztp[:])
                            else:
                                nc.scalar.copy(out=dst, in_=ztp[:])
                    hd = phd.tile([P, 512], F32, tag="hd")
                    for s in range(4):
                        nc.tensor.matmul(out=hd[:, :cw],
                                         lhsT=w1x_sb[:, s * P:(s + 1) * P],
                                         rhs=ztg[:, s * cw:(s + 1) * cw],
                                         start=(s == 0), stop=(s == 3))
                    # + per-graph term (4 graphs per tile x 32 torsions), relu(. + b1)
                    hsl = hgT[:, b0 * 4: b0 * 4 + nb * 4]
                    hbc = bass.AP(hsl.tensor, hsl.offset, list(hsl.ap) + [[0, MAX_T]])
                    hdf = pzs.tile([P, 512], F32, tag="hdf")
                    nc.vector.tensor_add(
                        out=hdf[:, :cw].rearrange("p (q u) -> p q u", q=nb * 4),
                        in0=hd[:, :cw].rearrange("p (q u) -> p q u", q=nb * 4),
                        in1=hbc)
                    hdn16 = pzs.tile([P, 512], F16, tag="hdn16")
                    nc.scalar.activation(out=hdn16[:, :cw], in_=hdf[:, :cw],
                                         func=AF.Relu, bias=b1_sb[:, 0:1])
                    for bb in range(nb):
                        lg = plg.tile([P, ACTD], F32, tag="lg")
                        nc.tensor.matmul(out=lg[:], lhsT=hdn16[:, bb * P:(bb + 1) * P],
                                         rhs=w2_sb[:], start=True, stop=True)
                        lsb = pzs.tile([P, ACTD], F32, tag="lsb")
                        nc.vector.tensor_add(out=lsb[:], in0=lg[:], in1=b2_sb[:])
                        nc.sync.dma_start(
                            out=out[(b0 + bb) * P:(b0 + bb + 1) * P, :],
                            in_=lsb[:])
    nc.compile()
    return nc


def _host_prep(inputs):
    x = np.asarray(inputs["x"], np.float32)
    nonring = np.asarray(inputs["nonring"], np.int32)
    w_ih = np.asarray(inputs["w_ih"], np.float32)
    w_hh = np.asarray(inputs["w_hh"], np.float32)
    b_ih = np.asarray(inputs["b_ih"], np.float32)
    b_hh = np.asarray(inputs["b_hh"], np.float32)
    fc_w = np.asarray(inputs["fc_w"], np.float32)
    fc_b = np.asarray(inputs["fc_b"], np.float32)
    mlp_w1 = np.asarray(inputs["mlp_w1"], np.float32)
    mlp_b1 = np.asarray(inputs["mlp_b1"], np.float32)
    mlp_w2 = np.asarray(inputs["mlp_w2"], np.float32)
    mlp_b2 = np.asarray(inputs["mlp_b2"], np.float32)

    fcwT = fc_w.T
    w1T = mlp_w1.T
    # tanh-only LSTM: wA/fcq absorb the 0.5 of H=2h; i/f/o biases halved for
    # sigmoid(x) = (1+tanh(x/2))/2
    bsum = (b_ih + b_hh).reshape(4, P).T.copy()
    bsum[:, 0] *= 0.5
    bsum[:, 1] *= 0.5
    bsum[:, 3] *= 0.5
    p = np.arange(P).reshape(P, 1)
    lower = p < NODES_PER_G
    expbias = np.where(np.concatenate([lower, ~lower], axis=1), -11.0, -1e30)
    rep = {
        "xfull16": np.ascontiguousarray(x.astype(np.float16)),
        "wA16": np.ascontiguousarray(
            (0.5 * (w_ih[:, :DIM] + w_hh)).T.astype(np.float16)),
        "wB16": np.ascontiguousarray(w_ih[:, DIM:].T.astype(np.float16)),
        "bs4": np.ascontiguousarray(bsum),
        "expb": np.ascontiguousarray(expbias.astype(np.float32)),
        "fcq16": np.ascontiguousarray((0.5 * fcwT[:DIM]).astype(np.float16)),
        "fcr16": np.ascontiguousarray(fcwT[DIM:].astype(np.float16)),
        "fcb": np.ascontiguousarray(fc_b.reshape(P, 1)),
        "w1g16": np.ascontiguousarray(w1T[:DIM].astype(np.float16)),
        "w1x16": np.ascontiguousarray(
            w1T[DIM:].reshape(4, P, DIM).transpose(1, 0, 2).reshape(P, 4 * DIM)
            .astype(np.float16)),
        "b1": np.ascontiguousarray(mlp_b1.reshape(P, 1)),
        "w2T16": np.ascontiguousarray(mlp_w2.T.astype(np.float16)),
        "b2t": np.ascontiguousarray(np.tile(mlp_b2.reshape(1, ACTD), (P, 1))),
        "ones16d": np.ones((P, P), np.float16),
        "ident16": np.eye(P, dtype=np.float16),
    }

    in_maps = []
    for k in range(NC):
        xl = x[k * NN:(k + 1) * NN]
        nr = nonring[k * NT:(k + 1) * NT]
        m = dict(rep)
        m["xT16"] = np.ascontiguousarray(xl.T.astype(np.float16))
        m["xrm16"] = np.ascontiguousarray(
            xl.reshape(NTILE, P, DIM).transpose(1, 0, 2).reshape(P, NN)
            .astype(np.float16))
        m["idx4"] = np.ascontiguousarray(
            nr.reshape(TTILE, P, 4).transpose(1, 0, 2).reshape(P, 4 * TTILE))
        in_maps.append(m)
    return in_maps


def kernel(**inputs) -> np.ndarray:
    global LAST, _CACHED
    if _CACHED is None:
        _CACHED = build_bass()
    nc = _CACHED
    in_maps = _host_prep(inputs)
    LAST = run_bass_kernel_spmd(nc, in_maps, core_ids=list(range(NC)))
    outs = [LAST.results[k]["out"].reshape(NG, MAX_T, ACTD) for k in range(NC)]
    return np.concatenate(outs, axis=0)


if __name__ == "__main__":
    nc = build_bass()
    print("build ok")
